# revision 22
# baseline (speedup 1.0000x reference)
"""Trainium2 Bass kernel for BertSelfShiftedLocalAttention — v5.

Problem (hardcoded): B=4, S=8256, H=768, NH=12, HD=64, W=128, SHIFT=64.
  head  = full attention over tokens [0:64) with RoPE positions 0..63
  body  = 64 independent windows of 128 tokens, window-local RoPE 0..127

Sharding: 2 cores per batch element (core 2b: windows 0..31; core 2b+1:
windows 32..63 of that batch); the 64-token shifted head block is computed
on the host in fp32.

v5 design (vs v2's per-window N=128 pipeline):
  - All projections bf16 over 512-token macro-tiles: N=512 moving-dim
    matmuls (6 accumulating K-chunks per feature tile) instead of N=128
    per-window ones; amortizes per-MM (LDWEIGHTS/dispatch) overhead.
    fp8 DoubleRow was evaluated and rejected: e4m3 Q/K quantization alone
    costs rel err ~0.027 > the 2e-2 gate (score jitter is not softmax-
    damped enough).
  - V projection bf16 token-major per window (X chunk stationary, Wv
    moving, N=512+256), as in v2.
  - Scores read even heads from qka partitions 0:64 and odd heads from a
    qkhi partner-copy (one [64, 12*TILE] DMA per macro-tile instead of per
    window). Reading partitions 64:128 directly as matmul operands crashes
    this hardware (quadrant-3 xbus limitation).
  - Softmax normalization on host via the ones-column trick: PV matmul
    appends a ones column per head so row-sums land in col 64; device
    emits raw [ctx|Z] (12 heads x 65 cols, bf16), host divides.
  - PSUM plan (slots are bank-padded): qk 3x1 + v 2 + sc 2 + ctx 1 = 8
    banks.
  - Rejected by measurement: interleaving pairs of accumulation chains
    (j-pairs for Q/K, +5us) and fusing the two ctx halves into one 2-bank
    psum tile (+12us with qk bufs=2).
"""
import numpy as np
import ml_dtypes

import concourse.bacc as bacc
import concourse.bass as bass
import concourse.tile as tile
from concourse import mybir
from concourse.bass_utils import run_bass_kernel_spmd

BF16 = ml_dtypes.bfloat16
F32 = mybir.dt.float32
BF = mybir.dt.bfloat16
Copy = mybir.ActivationFunctionType.Copy
Exp = mybir.ActivationFunctionType.Exp

B, S, H = 4, 8256, 768
NH, HD = 12, 64
W, SHIFT = 128, 64
NCORES = 8
TBODY = 4096          # tokens per core
TILE = 512            # projection macro-tile (4 windows)
NTILES = TBODY // TILE  # 8
WPT = TILE // W       # 4 windows per tile
NW = TBODY // W       # 32 windows per core

# Score matmuls for odd heads read q/k at partitions 64:128 directly (PE row
# groups 2-3) — CRASHES on this hardware (quadrant-3 xbus bug: streaming the
# moving operand into rows 64:127 is not supported). Keep False: DMA-copy
# partitions 64:128 down to a second buffer as v2 did.
USE_P64 = False

# Emit a standalone LDWEIGHTS before each projection matmul and mark the
# matmul non-self-loading, so the PE's reorder window can pull the weight
# load ahead of the in-flight matmul stream.
EXPLICIT_LDW = True


def _build_program(has_bias=False, loop_n=None, use_p64=None, parts="proj,v,sc,ctx"):
    from contextlib import ExitStack, nullcontext

    if use_p64 is None:
        use_p64 = USE_P64
    parts = set(parts.split(","))
    do_v = "v" in parts
    do_sc = "sc" in parts
    do_ctx = "ctx" in parts and do_sc
    nc = bacc.Bacc(None, target_bir_lowering=False, debug=False)

    xtb = nc.dram_tensor("xtb", [128, NTILES * 6 * TILE], BF, kind="ExternalInput")
    wq = nc.dram_tensor("wq", [H, H], BF, kind="ExternalInput")
    wk = nc.dram_tensor("wk", [H, H], BF, kind="ExternalInput")
    wv = nc.dram_tensor("wv", [H, H], BF, kind="ExternalInput")
    cosb = nc.dram_tensor("cosb", [128, TILE], BF, kind="ExternalInput")
    sinb = nc.dram_tensor("sinb", [128, TILE], BF, kind="ExternalInput")
    if has_bias:
        bqkr = nc.dram_tensor("bqkr", [128, 12 * TILE], BF, kind="ExternalInput")
    out = nc.dram_tensor("out", [TBODY, 780], BF, kind="ExternalOutput")

    with tile.TileContext(nc) as tc, ExitStack() as es:
        consts = es.enter_context(tc.tile_pool(name="consts", bufs=1))
        xb_pool = es.enter_context(tc.tile_pool(name="xb", bufs=3))
        qka_pool = es.enter_context(tc.tile_pool(name="qka", bufs=2))
        tmp_pool = es.enter_context(tc.tile_pool(name="tmp", bufs=2))
        qsw_pool = es.enter_context(tc.tile_pool(name="qsw", bufs=2))
        exp_pool = es.enter_context(tc.tile_pool(name="expp", bufs=3))
        cs_pool = es.enter_context(tc.tile_pool(name="cs", bufs=2))
        qkhi_pool = es.enter_context(tc.tile_pool(name="qkhi", bufs=2))
        pp_qk = es.enter_context(tc.tile_pool(name="pp_qk", bufs=3, space="PSUM"))
        pp_v = es.enter_context(tc.tile_pool(name="pp_v", bufs=1, space="PSUM"))
        pp_sc = es.enter_context(tc.tile_pool(name="pp_sc", bufs=1, space="PSUM"))
        pp_ctx = es.enter_context(tc.tile_pool(name="pp_ctx", bufs=1, space="PSUM"))

        # resident constants
        wq_sb = consts.tile([128, 6, H], BF, tag="wq")
        wk_sb = consts.tile([128, 6, H], BF, tag="wk")
        wv_sb = consts.tile([128, 6, H], BF, tag="wv")
        nc.sync.dma_start(out=wq_sb, in_=wq.rearrange("(i p) o -> p i o", p=128))
        nc.gpsimd.dma_start(out=wk_sb, in_=wk.rearrange("(i p) o -> p i o", p=128))
        nc.gpsimd.dma_start(out=wv_sb, in_=wv.rearrange("(i p) o -> p i o", p=128))
        cos_sb = consts.tile([128, TILE], BF, tag="cosb")
        sin_sb = consts.tile([128, TILE], BF, tag="sinb")
        nc.sync.dma_start(out=cos_sb, in_=cosb[:, :])
        nc.sync.dma_start(out=sin_sb, in_=sinb[:, :])
        if has_bias:
            bqkr_sb = consts.tile([128, 12 * TILE], BF, tag="bqkr")
            nc.sync.dma_start(out=bqkr_sb, in_=bqkr[:, :])

        # three V buffers with preset ones-columns (evictions write cols 0:64)
        v_sbufs = [
            consts.tile([128, 12 * 66], BF, tag=f"v_sb{i}", name=f"v_sb{i}")
            for i in range(3)
        ]
        v65s = [t.rearrange("p (h c) -> p h c", c=66) for t in v_sbufs]
        for v65 in v65s:
            if do_v:
                nc.gpsimd.memset(v65[:, :, 64:66], 1.0)
            else:
                nc.gpsimd.memset(v65[:, :, :], 1.0)

        xbt, st = {}, {}

        def fetch(g, eng=None):
            tb = xb_pool.tile([128, 6 * TILE], BF, tag="xbt", name="xbt")
            (eng or nc.gpsimd).dma_start(
                out=tb, in_=xtb[:, g * 6 * TILE : (g + 1) * 6 * TILE]
            )
            xbt[g] = tb.rearrange("p (i t) -> p i t", t=TILE)

        def emit_proj(g):
            # Q then K projection for tile g (bf16 N=512) + RoPE chain.
            xg = xbt[g]
            qka = qka_pool.tile([128, 12 * TILE], BF, tag="qka", name="qka")
            st[("qka", g)] = qka
            ropes = []
            for proj in range(2):
                w_sb = wq_sb if proj == 0 else wk_sb
                tmp = tmp_pool.tile([128, 6 * TILE], BF, tag="tmp", name="tmp")
                qsw = qsw_pool.tile([128, 6 * TILE], BF, tag="qsw", name="qsw")
                s0 = proj * 6 * TILE
                for j in range(6):
                    ps = pp_qk.tile([128, 512], F32, tag="qk_ps", name="qk_ps")
                    for i in range(6):
                        wsl = w_sb[:, i, 128 * j : 128 * (j + 1)]
                        if EXPLICIT_LDW:
                            nc.tensor.ldweights(wsl)
                        r = nc.tensor.matmul(
                            ps[:, :],
                            lhsT=wsl,
                            rhs=xg[:, i, :],
                            start=(i == 0),
                            stop=(i == 5),
                        )
                        if EXPLICIT_LDW:
                            getattr(r, "ins", r).ldweights = False
                    sec = s0 + j * TILE
                    if proj == 0:
                        nc.scalar.activation(
                            out=qka[:, sec : sec + TILE], in_=ps[:, :], func=Copy
                        )
                    else:
                        nc.vector.tensor_copy(qka[:, sec : sec + TILE], ps[:, :])
                    nc.vector.tensor_mul(
                        tmp[:, j * TILE : (j + 1) * TILE],
                        qka[:, sec : sec + TILE],
                        cos_sb[:, :],
                    )
                # partner-swap copies for rotate-half (partition block swaps)
                for eng, (a, b2) in zip(
                    (nc.sync, nc.sync, nc.gpsimd, nc.gpsimd),
                    ((0, 32), (32, 0), (64, 96), (96, 64)),
                ):
                    eng.dma_start(
                        out=qsw[a : a + 32, :], in_=qka[b2 : b2 + 32, s0 : s0 + 6 * TILE]
                    )
                ropes.append((tmp, qsw, s0))
            for tmp, qsw, s0 in ropes:
                for j in range(6):
                    jc = j * TILE
                    sec = s0 + jc
                    nc.vector.tensor_mul(
                        qsw[:, jc : jc + TILE], qsw[:, jc : jc + TILE], sin_sb[:, :]
                    )
                    nc.vector.tensor_add(
                        qka[:, sec : sec + TILE], tmp[:, jc : jc + TILE],
                        qsw[:, jc : jc + TILE],
                    )
                    if has_bias:
                        nc.vector.tensor_add(
                            qka[:, sec : sec + TILE],
                            qka[:, sec : sec + TILE],
                            bqkr_sb[:, sec : sec + TILE],
                        )
            if not use_p64 and do_sc:
                qkhi = qkhi_pool.tile([64, 12 * TILE], BF, tag="qkhi", name="qkhi")
                st[("qkhi", g)] = qkhi
                nc.sync.dma_start(out=qkhi[0:64, :], in_=qka[64:128, :])

        def emit_sc(w, half):
            # scores for 6 heads: even head at rows 0:64, odd at 64:128
            g, wi = w // WPT, w % WPT
            qka = st[("qka", g)]
            scps = pp_sc.tile([128, 768], F32, tag="sc_ps", name="sc_ps")
            st[("sc", w, half)] = scps
            for jj in range(3):
                j = half * 3 + jj
                qcol = j * TILE + wi * 128
                kcol = (6 + j) * TILE + wi * 128
                for p in range(2):
                    if p == 0 or use_p64:
                        src, p0 = qka, 64 * p
                    else:
                        src, p0 = st[("qkhi", g)], 0
                    nc.tensor.matmul(
                        scps[:, (jj * 2 + p) * 128 : (jj * 2 + p + 1) * 128],
                        lhsT=src[p0 : p0 + 64, kcol : kcol + 128],
                        rhs=src[p0 : p0 + 64, qcol : qcol + 128],
                        start=True,
                        stop=True,
                    )

        def emit_exp(w, half):
            scps = st.pop(("sc", w, half))
            exp_sb = st[("exp", w)]
            nc.scalar.activation(
                out=exp_sb[:, half * 768 : (half + 1) * 768],
                in_=scps[:, 0:768],
                func=Exp,
            )

        def emit_v(w):
            g, wi = w // WPT, w % WPT
            xb = xbt[g]
            vps = pp_v.tile([128, 768], F32, tag="v_ps", name="v_ps")
            st[("vps", w)] = vps
            for i in range(6):
                for c0, c1 in ((0, 512), (512, 768)):
                    nc.tensor.matmul(
                        vps[:W, c0:c1],
                        lhsT=xb[:, i, wi * 128 : (wi + 1) * 128],
                        rhs=wv_sb[:, i, c0:c1],
                        start=(i == 0),
                        stop=(i == 5),
                    )

        def emit_evv(w):
            vps = st.pop(("vps", w))
            v65w = v65s[w % 3]
            nc.scalar.activation(
                out=v65w[:W, :, 0:64],
                in_=vps[:W, 0:768].rearrange("p (h d) -> p h d", d=64),
                func=Copy,
            )

        def emit_ctx(u, half, cs):
            exp_sb = st[("exp", u)]
            cps = pp_ctx.tile([128, 512], F32, tag="ctx_ps", name="ctx_ps")
            for hh in range(6):
                h = half * 6 + hh
                nc.tensor.matmul(
                    cps[:W, hh * 65 : (hh + 1) * 65],
                    lhsT=exp_sb[:, h * 128 : (h + 1) * 128],
                    rhs=v65s[u % 3][:, h, 0:65],
                    start=True,
                    stop=True,
                )
            nc.vector.tensor_copy(
                cs[:, half * 390 : (half + 1) * 390], cps[:W, 0:390]
            )

        loop_cm = tc.For_i(0, loop_n, 1) if loop_n else nullcontext()
        with loop_cm:
            fetch(0, eng=nc.scalar)
            fetch(1, eng=nc.scalar)
            emit_proj(0)
            for w in range(NW + 1):
                g = w // WPT
                if w >= 1 and do_ctx:
                    u = w - 1
                    cs = cs_pool.tile([128, 780], BF, tag="cs", name="cs")
                    emit_ctx(u, 0, cs)
                    emit_ctx(u, 1, cs)
                    nc.sync.dma_start(out=out[u * W : (u + 1) * W, :], in_=cs[:, :])
                    st.pop(("exp", u))
                if w % WPT == 0 and w < NW:
                    gn = g + 1
                    if gn < NTILES:
                        if gn + 1 < NTILES:
                            fetch(gn + 1)
                        emit_proj(gn)
                if w < NW:
                    if do_sc:
                        exp_sb = exp_pool.tile([128, 12 * 128], BF, tag="exp", name="exp_sb")
                        st[("exp", w)] = exp_sb
                        emit_sc(w, 0)
                        emit_exp(w, 0)
                    if do_v:
                        emit_v(w)
                        emit_evv(w)
                    if do_sc:
                        emit_sc(w, 1)
                        emit_exp(w, 1)
            xbt.clear()
            st.clear()

    return nc


def _rope_tables512():
    # [128, 12*TILE]: sections 0..5 = Q feature tiles, 6..11 = K; each
    # section = the window-local [128,128] table tiled across the 4 windows
    # of a macro-tile.
    m = np.arange(32)
    f = 1.0 / (10000.0 ** (2.0 * m / HD))
    ang = np.outer(f, np.arange(W))  # [32, 128]
    c = np.tile(np.cos(ang), (4, 1))  # [128, 128]
    s = np.tile(np.sin(ang), (4, 1))
    sgn = np.where((np.arange(128) % 64) < 32, -1.0, 1.0)[:, None]
    s = s * sgn
    c4 = np.tile(c, (1, WPT))  # [128, TILE]
    s4 = np.tile(s, (1, WPT))
    return c4.astype(BF16), s4.astype(BF16)


def _rope_bias(bias, tw):
    # RoPE of a position-independent bias vector, in [o-tile partition, t] layout.
    m = np.arange(32)
    f = 1.0 / (10000.0 ** (2.0 * m / HD))
    pos = np.arange(tw)
    ang = np.outer(f, pos)
    c = np.tile(np.cos(ang), (4, 1))  # [128, tw]
    s = np.tile(np.sin(ang), (4, 1))
    sgn = np.where((np.arange(128) % 64) < 32, -1.0, 1.0)[:, None]
    blocks = []
    bo = bias.reshape(6, 128)
    for j in range(6):
        bj = bo[j][:, None]
        p = np.arange(128)
        swap_idx = np.where((p % 64) < 32, p + 32, p - 32)
        bswap = bo[j][swap_idx][:, None]
        blocks.append(bj * c + bswap * (s * sgn))
    return np.concatenate(blocks, axis=1)  # [128, 6*tw]


def _bias_table512(bq8, bk):
    def sec(bias):
        rb = _rope_bias(bias, W)  # [128, 6*128]
        return np.concatenate(
            [np.tile(rb[:, j * W : (j + 1) * W], (1, WPT)) for j in range(6)], axis=1
        )

    return np.concatenate([sec(bq8), sec(bk)], axis=1).astype(BF16)


_PROGRAMS = {}


def _get_program(has_bias):
    key = has_bias
    if key not in _PROGRAMS:
        nc = _build_program(has_bias=has_bias)
        nc.finalize()
        _PROGRAMS[key] = nc
    return _PROGRAMS[key]


def _make_in_maps(inputs):
    hs = np.asarray(inputs["hidden_states"], np.float32)
    Wq = np.asarray(inputs["Wq"], np.float32)
    Wk = np.asarray(inputs["Wk"], np.float32)
    Wv = np.asarray(inputs["Wv"], np.float32)
    bq = np.asarray(inputs["bq"], np.float32)
    bk = np.asarray(inputs["bk"], np.float32)
    bv = np.asarray(inputs["bv"], np.float32)
    has_bias = bool(np.any(bq) or np.any(bk) or np.any(bv))

    consts = {
        "wq": np.ascontiguousarray((Wq / 8.0).T).astype(BF16),
        "wk": np.ascontiguousarray(Wk.T).astype(BF16),
        "wv": np.ascontiguousarray(Wv.T).astype(BF16),
    }
    consts["cosb"], consts["sinb"] = _rope_tables512()
    if has_bias:
        consts["bqkr"] = _bias_table512(bq / 8.0, bk)

    in_maps = []
    for c in range(NCORES):
        b, half = c // 2, c % 2
        xs = hs[b, SHIFT + half * TBODY : SHIFT + (half + 1) * TBODY, :]
        xt = (
            xs.T.reshape(6, 128, NTILES, TILE)
            .transpose(1, 2, 0, 3)
            .reshape(128, NTILES * 6 * TILE)
        )
        in_maps.append({**consts, "xtb": np.ascontiguousarray(xt).astype(BF16)})
    return in_maps


def _head_block(hs, Wq, bq, Wk, bk, Wv, bv):
    # fp32 host attention over the first SHIFT tokens of each batch
    L = SHIFT
    inv = 1.0 / (10000.0 ** (np.arange(0, HD, 2, dtype=np.float32) / np.float32(HD)))
    ang = np.arange(L, dtype=np.float32)[:, None] * inv[None, :]
    cos, sin = np.cos(ang)[None, :, None, :], np.sin(ang)[None, :, None, :]
    x = hs[:, :L, :].astype(np.float32)
    qh = (x @ Wq.T + bq).reshape(B, L, NH, HD)
    kh = (x @ Wk.T + bk).reshape(B, L, NH, HD)
    vh = (x @ Wv.T + bv).reshape(B, L, NH, HD)

    def rope(z):
        z1, z2 = z[..., : HD // 2], z[..., HD // 2 :]
        return np.concatenate([z1 * cos - z2 * sin, z2 * cos + z1 * sin], -1)

    qh, kh = rope(qh), rope(kh)
    sc = np.einsum("blhd,bmhd->bhlm", qh, kh) / np.float32(np.sqrt(HD))
    sc = sc - sc.max(-1, keepdims=True)
    p = np.exp(sc)
    p = p / p.sum(-1, keepdims=True)
    return np.einsum("bhlm,bmhd->blhd", p, vh).reshape(B, L, H)


def kernel(hidden_states, attention_mask, Wq, bq, Wk, bk, Wv, bv):
    inputs = {
        "hidden_states": hidden_states, "Wq": Wq, "Wk": Wk, "Wv": Wv,
        "bq": bq, "bk": bk, "bv": bv,
    }
    has_bias = bool(
        np.any(np.asarray(bq)) or np.any(np.asarray(bk)) or np.any(np.asarray(bv))
    )
    in_maps = _make_in_maps(inputs)
    nc = _get_program(has_bias)
    res = run_bass_kernel_spmd(nc, in_maps, list(range(NCORES)))

    outp = np.empty((B, S, H), np.float32)
    bvf = np.asarray(bv, np.float32)
    outp[:, :SHIFT, :] = _head_block(
        np.asarray(hidden_states, np.float32),
        np.asarray(Wq, np.float32), np.asarray(bq, np.float32),
        np.asarray(Wk, np.float32), np.asarray(bk, np.float32),
        np.asarray(Wv, np.float32), np.asarray(bv, np.float32),
    )
    for c in range(NCORES):
        r = np.asarray(res.results[c]["out"], dtype=np.float32)  # [TBODY, 780]
        r3 = r.reshape(TBODY, 12, 65)
        full = (r3[..., 0:64] / r3[..., 64:65]).reshape(TBODY, H)
        if has_bias:
            full = full + bvf[None, :]
        b, half = c // 2, c % 2
        t0 = SHIFT + half * TBODY
        outp[b, t0 : t0 + TBODY] = full
    return outp
